# revision 33
# baseline (speedup 1.0000x reference)
"""Trainium2 Bass kernel for the DVAE problem.

Full-input contract: kernel(**inputs) takes the complete (unsharded) numpy
inputs and returns (node_logits, edge_probs, mu, logvar) as full numpy arrays.
Internally shards batch 64 -> 8 cores (data parallel), weights replicated.

Math (see reference):
  h      = relu(x @ W1 + b1) @ W2 + b2          [B,N,H]
  hp     = mean(h, axis=1)                      [B,H]
  mu/lv  = hp @ {mu_w,lv_w} + {mu_b,lv_b}       [B,L]
  z      = mu + eps * exp(0.5 lv)
  hd     = relu(z @ z_w + z_b)                  [B,H]
  nlrow  = hd @ nd_w + nd_b                     [B,T]  (broadcast over N)
  e      = sigmoid(relu([hd,hd] @ e1_w + e1_b) @ e2_w + e2_b)   [B]
  edge   = e[:,None,None] * strict_lower(N)     [B,N,N]

Key restructurings:
  - adj_matrix is unused by the math: never shipped or read.
  - mean commutes with the second GNN linear: pool relu(xW1+b1) first, then
    a tiny [B,H]x[H,H] matmul. The 1/N mean is folded into the relu's scale
    (relu is positively homogeneous), with b1 pre-scaled by 1/N.
  - GEMM computed transposed ([h, rows] tiles) so bias is per-partition and
    the node-pool is a free ACT accum_out (relu written in place to PSUM
    and discarded).
  - All matmuls run in fp32r: 1 cyc/row at N=512 vs 4 for fp32 (no HI/LO
    double pass), with operands rounded on load (cast-DMA) or by the
    producing compute op, as the BIR verifier requires.
  - edge_probs is split into an upper-triangle ZEROS pre-pass (independent
    of e, written during the load ramp) and ragged lower-trapezoid writes
    after each decoder chain, shrinking the serial tail.
  - Decoder chain (per 4 batches, rhs N=4) and edge writes of quad q are
    interleaved between quad q+1's GEMM windows so the in-order PE/DVE
    streams fill each other's dependency gaps.
"""

import sys

sys.path.insert(0, "/opt/trn_rl_repo")

from contextlib import ExitStack

import ml_dtypes
import numpy as np

import concourse.bass as bass
from concourse import bacc
import concourse.mybir as mybir
import concourse.tile as tile
from concourse.bass import ds
from concourse.bass_utils import run_bass_kernel_spmd
from concourse.masks import make_identity

F32 = mybir.dt.float32
F32R = mybir.dt.float32r
BF16 = mybir.dt.bfloat16
AF = mybir.ActivationFunctionType

B, N, H, L, T = 64, 1024, 512, 256, 32
NCORES = 8
BPC = B // NCORES  # batches per core = 8
NQUADS = BPC // 4  # chain granularity: 4 batches


def _build():
    nc = bacc.Bacc(None, target_bir_lowering=False)

    x_i = nc.declare_dram_parameter("node_features", [H, BPC * N], BF16, isOutput=False)
    eps_i = nc.declare_dram_parameter("eps", [BPC, L], F32, isOutput=False)
    w1_i = nc.declare_dram_parameter("gnn_w1", [H, H], F32, isOutput=False)
    b1_i = nc.declare_dram_parameter("gnn_b1", [H], F32, isOutput=False)
    w2_i = nc.declare_dram_parameter("gnn_w2", [H, H], F32, isOutput=False)
    b2_i = nc.declare_dram_parameter("gnn_b2", [H], F32, isOutput=False)
    muw_i = nc.declare_dram_parameter("mu_w", [H, L], F32, isOutput=False)
    mub_i = nc.declare_dram_parameter("mu_b", [L], F32, isOutput=False)
    lvw_i = nc.declare_dram_parameter("lv_w", [H, L], F32, isOutput=False)
    lvb_i = nc.declare_dram_parameter("lv_b", [L], F32, isOutput=False)
    zw_i = nc.declare_dram_parameter("z_w", [L, H], F32, isOutput=False)
    zb_i = nc.declare_dram_parameter("z_b", [H], F32, isOutput=False)
    ndw_i = nc.declare_dram_parameter("nd_w", [H, T], F32, isOutput=False)
    ndb_i = nc.declare_dram_parameter("nd_b", [T], F32, isOutput=False)
    e1w_i = nc.declare_dram_parameter("e1_w", [2 * H, H], F32, isOutput=False)
    e1b_i = nc.declare_dram_parameter("e1_b", [H], F32, isOutput=False)
    e2w_i = nc.declare_dram_parameter("e2_w", [H, 1], F32, isOutput=False)
    e2b_i = nc.declare_dram_parameter("e2_b", [1], F32, isOutput=False)

    nl_o = nc.declare_dram_parameter("node_logits", [BPC, N, T], F32, isOutput=True)
    ep_o = nc.declare_dram_parameter("edge_probs", [BPC, N, N], F32, isOutput=True)
    mu_o = nc.declare_dram_parameter("mu", [BPC, L], F32, isOutput=True)
    lv_o = nc.declare_dram_parameter("logvar", [BPC, L], F32, isOutput=True)


    with tile.TileContext(nc) as tc, ExitStack() as ctx:
        singles = ctx.enter_context(tc.tile_pool(name="singles", bufs=1))
        xTw = ctx.enter_context(tc.tile_pool(name="xTw", bufs=3))
        epool = ctx.enter_context(tc.tile_pool(name="epool", bufs=3))
        chpool = ctx.enter_context(tc.tile_pool(name="chain", bufs=4))
        outp = ctx.enter_context(tc.tile_pool(name="outp", bufs=2))
        psA = ctx.enter_context(tc.tile_pool(name="psA", bufs=4, space="PSUM"))
        psC = ctx.enter_context(tc.tile_pool(name="psC", bufs=2, space="PSUM"))

        # ---- critical-path prologue: W1, b1, first x windows ----
        w1sb = singles.tile([128, 4, H], BF16)
        nc.gpsimd.dma_start(out=w1sb, in_=w1_i[:, :].rearrange("(c p) h -> p c h", p=128))
        b1sb = singles.tile([128, 4], F32)
        nc.scalar.dma_start(out=b1sb, in_=b1_i[:].rearrange("(c p) -> p c", p=128))
        nc.scalar.mul(b1sb, b1sb, 1.0 / N)  # relu scale folds the 1/N mean
        # PE warm-up: dummy matmuls over the just-loaded W1 keep the HAM
        # activity window busy while x streams in, so real GEMM work starts
        # at 2.4 GHz instead of the cold 1.2 GHz.
        pswarm = psA.tile([128, 512], F32, tag="psa")
        for i in range(16):
            nc.tensor.matmul(
                pswarm,
                w1sb[:, i % 4, ds(0, 128)],
                w1sb[:, i % 4, :],
                start=True,
                stop=True,
            )

        # masks/zeros memsets early (DVE is idle during the ramp)
        masks = singles.tile([128, 8, N], BF16)
        nc.vector.memset(masks, 1.0)
        ones128 = singles.tile([128, 128], F32)
        nc.vector.memset(ones128, 1.0)

        # persistent accumulators / staging (declared early, used throughout)
        accall = singles.tile([128, NG := 16, 4], F32)
        muall = singles.tile([128, 2, BPC], F32)
        lvall = singles.tile([128, 2, BPC], F32)
        ndall = singles.tile([128, 32], F32)
        eall = singles.tile([128, BPC], F32)

        # ---- deferred bulk prologue (emitted between early GEMM windows) ----
        def emit_bulk_prologue():
            nonlocal w2sb, muwsb, lvwsb, zwsb, ndwsb, e1wsb, e2cols
            nonlocal b2sb, mubsb, lvbsb, zbsb, e1bsb, ndbsb, e2bsb, epsT
            nonlocal e2rep, selT, ident
            e2rep = singles.tile([128, 4, 128], F32R)
            selT = singles.tile([128, 128], F32R)
            ident = singles.tile([128, 128], F32)
            w2sb = singles.tile([128, 4, H], F32R)
            nc.gpsimd.dma_start(
                out=w2sb, in_=w2_i[:, :].rearrange("(c p) h -> p c h", p=128)
            )
            muwsb = singles.tile([128, 4, L], F32R)
            nc.gpsimd.dma_start(
                out=muwsb, in_=muw_i[:, :].rearrange("(c p) l -> p c l", p=128)
            )
            lvwsb = singles.tile([128, 4, L], F32R)
            nc.gpsimd.dma_start(
                out=lvwsb, in_=lvw_i[:, :].rearrange("(c p) l -> p c l", p=128)
            )
            zwsb = singles.tile([128, 2, H], F32R)
            nc.gpsimd.dma_start(
                out=zwsb, in_=zw_i[:, :].rearrange("(c p) h -> p c h", p=128)
            )
            ndwsb = singles.tile([128, 4, T], F32R)
            nc.gpsimd.dma_start(
                out=ndwsb, in_=ndw_i[:, :].rearrange("(c p) t -> p c t", p=128)
            )
            e1wsb = singles.tile([128, 8, H], F32R)
            nc.gpsimd.dma_start(
                out=e1wsb, in_=e1w_i[:, :].rearrange("(c p) h -> p c h", p=128)
            )
            b2sb = singles.tile([128, 4], F32)
            nc.scalar.dma_start(out=b2sb, in_=b2_i[:].rearrange("(c p) -> p c", p=128))
            mubsb = singles.tile([128, 2], F32)
            nc.scalar.dma_start(out=mubsb, in_=mub_i[:].rearrange("(c p) -> p c", p=128))
            lvbsb = singles.tile([128, 2], F32)
            nc.scalar.dma_start(out=lvbsb, in_=lvb_i[:].rearrange("(c p) -> p c", p=128))
            zbsb = singles.tile([128, 4], F32)
            nc.scalar.dma_start(out=zbsb, in_=zb_i[:].rearrange("(c p) -> p c", p=128))
            e1bsb = singles.tile([128, 4], F32)
            nc.scalar.dma_start(out=e1bsb, in_=e1b_i[:].rearrange("(c p) -> p c", p=128))
            ndbsb = singles.tile([128, 1], F32)
            nc.scalar.dma_start(
                out=ndbsb[0:T, :], in_=ndb_i[:].rearrange("(t o) -> t o", o=1)
            )
            e2bsb = singles.tile([128, 1], F32)
            nc.gpsimd.dma_start(
                out=e2bsb, in_=bass.AP(tensor=e2b_i, offset=0, ap=[[0, 128], [1, 1]])
            )
            epsT = singles.tile([128, 2, BPC], F32)
            for c in range(2):
                nc.scalar.dma_start(
                    out=epsT[:, c, :],
                    in_=eps_i[:, ds(128 * c, 128)].rearrange("b l -> l b"),
                )
            nc.vector.memset(ndall, 0.0)
            for r in range(8):
                nc.gpsimd.affine_select(
                    out=masks[:, r, :],
                    in_=masks[:, r, :],
                    compare_op=mybir.AluOpType.is_gt,
                    fill=0.0,
                    base=128 * r,
                    channel_multiplier=1,
                    pattern=[[-1, N]],
                )
            make_identity(nc, ident)
            # selection matrix: selT[b, p] = 1 iff p//16 == b
            selS = singles.tile([128, 128], F32)
            nc.vector.memset(selS, 1.0)
            nc.gpsimd.affine_select(
                out=selS[0:BPC, :], in_=selS[0:BPC, :],
                compare_op=mybir.AluOpType.is_ge, fill=0.0,
                base=0, channel_multiplier=-16, pattern=[[1, 128]],
            )
            nc.gpsimd.affine_select(
                out=selS[0:BPC, :], in_=selS[0:BPC, :],
                compare_op=mybir.AluOpType.is_gt, fill=0.0,
                base=16, channel_multiplier=16, pattern=[[-1, 128]],
            )
            nc.vector.tensor_copy(selT[0:BPC, :], selS[0:BPC, :])
            # e2 weight replicated across all 128 stationary columns so the
            # final sigmoid lands on every partition (no partition broadcast)
            e2f = singles.tile([128, 4], F32)
            nc.scalar.dma_start(
                out=e2f, in_=e2w_i[:, :].rearrange("(c p) o -> p (c o)", p=128)
            )
            for k in range(4):
                nc.vector.tensor_scalar_mul(
                    e2rep[:, k, :], ones128, e2f[:, ds(k, 1)]
                )

        w2sb = muwsb = lvwsb = zwsb = ndwsb = e1wsb = e2cols = None
        b2sb = mubsb = lvbsb = zbsb = e1bsb = ndbsb = e2bsb = epsT = None
        e2rep = selT = ident = None

        # GEMM window: 1024 rows (2 row-groups); xT loaded directly from
        # the host-transposed input, then W1 matmuls with fused
        # relu(+bias, +1/N)+row-sum into accall.
        def emit_gemm_window(q, hl):
            rbase = 4096 * q + 1024 * hl
            xT = xTw.tile([128, 4, 1024], BF16)
            nc.sync.dma_start(
                out=xT,
                in_=x_i[:, ds(rbase, 1024)].rearrange("(c p) r -> p c r", p=128),
            )
            for sub in range(2):
                gidx = 8 * q + 2 * hl + sub
                for m in range(4):
                    psa = psA.tile([128, 512], F32, tag="psa")
                    for c in range(4):
                        nc.tensor.matmul(
                            psa,
                            w1sb[:, c, ds(128 * m, 128)],
                            xT[:, c, ds(512 * sub, 512)],
                            start=(c == 0),
                            stop=(c == 3),
                        )
                    nc.scalar.activation(
                        out=psa,
                        in_=psa,
                        func=AF.Relu,
                        bias=b1sb[:, ds(m, 1)],
                        scale=1.0 / N,
                        accum_out=accall[:, gidx, ds(m, 1)],
                    )

        # Decoder chain for batches 4q..4q+3, split into two emission chunks
        # so GEMM windows can be slotted between them.
        def emit_chain_a(qb, nb):
            hp2 = chpool.tile([128, 4, 4], F32R, tag="hp2")
            for m in range(4):
                for j in range(nb):
                    g0 = 2 * (qb + j)
                    nc.vector.tensor_add(
                        hp2[:, m, ds(j, 1)],
                        accall[:, g0, ds(m, 1)],
                        accall[:, g0 + 1, ds(m, 1)],
                    )
            hpf = chpool.tile([128, 4, 4], F32R, tag="hpf")
            for m in range(4):
                ps = psC.tile([128, 4], F32, tag="chps")
                for k in range(4):
                    nc.tensor.matmul(
                        ps[:, 0:nb], w2sb[:, k, ds(128 * m, 128)], hp2[:, k, 0:nb],
                        start=(k == 0), stop=(k == 3),
                    )
                nc.vector.tensor_scalar_add(hpf[:, m, 0:nb], ps[:, 0:nb], b2sb[:, ds(m, 1)])
            for m in range(2):
                ps = psC.tile([128, 4], F32, tag="chps")
                for k in range(4):
                    nc.tensor.matmul(
                        ps[:, 0:nb], muwsb[:, k, ds(128 * m, 128)], hpf[:, k, 0:nb],
                        start=(k == 0), stop=(k == 3),
                    )
                nc.vector.tensor_scalar_add(
                    muall[:, m, ds(qb, nb)], ps[:, 0:nb], mubsb[:, ds(m, 1)]
                )
                ps2 = psC.tile([128, 4], F32, tag="chps")
                for k in range(4):
                    nc.tensor.matmul(
                        ps2[:, 0:nb], lvwsb[:, k, ds(128 * m, 128)], hpf[:, k, 0:nb],
                        start=(k == 0), stop=(k == 3),
                    )
                nc.vector.tensor_scalar_add(
                    lvall[:, m, ds(qb, nb)], ps2[:, 0:nb], lvbsb[:, ds(m, 1)]
                )
            return hpf

        def emit_chain_b(qb, nb):
            zT = chpool.tile([128, 2, 4], F32R, tag="zT")
            for m in range(2):
                tmp = chpool.tile([128, 4], F32, tag="ztmp")
                nc.scalar.activation(
                    out=tmp[:, 0:nb], in_=lvall[:, m, ds(qb, nb)], func=AF.Exp, scale=0.5
                )
                nc.vector.tensor_mul(tmp[:, 0:nb], tmp[:, 0:nb], epsT[:, m, ds(qb, nb)])
                nc.vector.tensor_add(
                    zT[:, m, 0:nb], tmp[:, 0:nb], muall[:, m, ds(qb, nb)]
                )
            hdT = chpool.tile([128, 4, 4], F32R, tag="hdT")
            for m in range(4):
                ps = psC.tile([128, 4], F32, tag="chps")
                for k in range(2):
                    nc.tensor.matmul(
                        ps[:, 0:nb], zwsb[:, k, ds(128 * m, 128)], zT[:, k, 0:nb],
                        start=(k == 0), stop=(k == 1),
                    )
                nc.scalar.activation(
                    out=hdT[:, m, 0:nb], in_=ps[:, 0:nb], func=AF.Relu,
                    bias=zbsb[:, ds(m, 1)],
                )
            return hdT

        def emit_chain_c(qb, nb, hdT):
            psn = psC.tile([128, 4], F32, tag="chps")
            for k in range(4):
                nc.tensor.matmul(
                    psn[0:T, 0:nb], ndwsb[:, k, :], hdT[:, k, 0:nb],
                    start=(k == 0), stop=(k == 3),
                )
            nc.vector.tensor_scalar_add(
                ndall[0:T, ds(qb, nb)], psn[0:T, 0:nb], ndbsb[0:T, :]
            )
            heT = chpool.tile([128, 4, 4], F32R, tag="heT")
            for m in range(4):
                ps = psC.tile([128, 4], F32, tag="chps")
                for k in range(8):
                    nc.tensor.matmul(
                        ps[:, 0:nb], e1wsb[:, k, ds(128 * m, 128)], hdT[:, k % 4, 0:nb],
                        start=(k == 0), stop=(k == 7),
                    )
                nc.scalar.activation(
                    out=heT[:, m, 0:nb], in_=ps[:, 0:nb], func=AF.Relu,
                    bias=e1bsb[:, ds(m, 1)],
                )
            pse = psC.tile([128, 4], F32, tag="chps")
            for k in range(4):
                nc.tensor.matmul(
                    pse[:, 0:nb], e2rep[:, k, :], heT[:, k, 0:nb],
                    start=(k == 0), stop=(k == 3),
                )
            nc.scalar.activation(
                out=eall[:, ds(qb, nb)], in_=pse[:, 0:nb], func=AF.Sigmoid, bias=e2bsb
            )

        # Lower-trapezoid edge writes for one batch: 256-row chunk k covers
        # columns [0, 256(k+1)) with mask-scaled values.
        def emit_edges_batch(bb, alt=False):
            for k in range(2):
                w = 512 * (k + 1)
                et = epool.tile([128, 4, w], F32, tag=f"et{k}")
                for h4 in range(4):
                    r = 4 * k + h4
                    nc.vector.tensor_scalar_mul(
                        et[:, h4, :],
                        masks[:, r, ds(0, w)],
                        eall[:, ds(bb, 1)],
                    )
                dma_eng = nc.gpsimd if (alt and k == 0) else nc.sync
                dma_eng.dma_start(
                    out=ep_o[bb, ds(512 * k, 512), ds(0, w)].rearrange(
                        "(t p) c -> p t c", p=128
                    ),
                    in_=et,
                )

        # ---------------- emission schedule ----------------
        # Chain granularity [4, 2, 2]: batches 4-5's chain only needs row
        # groups 8..11, so their edge writes overlap the last GEMM windows,
        # leaving just a 2-batch edge tail.
        emit_gemm_window(0, 0)
        emit_gemm_window(0, 1)
        emit_bulk_prologue()
        emit_gemm_window(0, 2)
        emit_gemm_window(0, 3)
        emit_gemm_window(1, 0)
        emit_chain_a(0, 4)
        emit_gemm_window(1, 1)
        hdT0 = emit_chain_b(0, 4)
        emit_chain_c(0, 4, hdT0)
        emit_gemm_window(1, 2)
        emit_edges_batch(0)
        emit_chain_a(4, 2)
        hdT1 = emit_chain_b(4, 2)
        emit_chain_c(4, 2, hdT1)
        emit_gemm_window(1, 3)
        emit_edges_batch(1)
        emit_edges_batch(2)
        emit_edges_batch(3)
        emit_edges_batch(4)
        emit_chain_a(6, 2)
        hdT2 = emit_chain_b(6, 2)
        emit_chain_c(6, 2, hdT2)
        emit_edges_batch(5)

        def emit_muv_out():
            # transpose [128 l, 2, 8 b] -> [8 b, 256 l] on the PE, then one
            # clean contiguous DMA per output (the direct scatter AP would
            # cost ~1024 4-byte descriptors and stall the SDMA engines).
            for src_t, dst in ((muall, mu_o), (lvall, lv_o)):
                pso = psC.tile([128, 2, 128], F32, tag="outT")
                for c in range(2):
                    nc.tensor.transpose(pso[0:BPC, c, :], src_t[:, c, :], ident)
                row = outp.tile([128, 2, 128], F32, tag="orow")
                nc.vector.tensor_copy(row[0:BPC, :, :], pso[0:BPC, :, :])
                nc.sync.dma_start(
                    out=dst[:, :], in_=row[0:BPC, :, :].rearrange("b c l -> b (c l)")
                )

        # ---------------- tail: node_logits + mu/logvar before last edges ----
        # nd rows land on all 128 partitions (p -> batch p//16) via a 0/1
        # selection matmul; no DRAM bounce needed.
        ndR = chpool.tile([128, 32], F32)
        nc.vector.transpose(ndR[0:32, :], ndall[0:32, :])  # [b, t] rows 0..7
        ndRr = chpool.tile([128, 32], F32R)
        nc.vector.tensor_copy(ndRr[0:BPC, :], ndR[0:BPC, :])
        psl = psA.tile([128, 512], F32, tag="psa")
        nc.tensor.matmul(
            psl[:, 0:T], selT[0:BPC, :], ndRr[0:BPC, :], start=True, stop=True
        )
        nlx = singles.tile([128, 64, T], F32)
        nc.vector.tensor_copy(
            out=nlx,
            in_=bass.AP(
                tensor=psl.tensor,
                offset=psl.offset,
                ap=[psl.ap[0], [0, 64], [1, T]],
            ),
        )
        nc.sync.dma_start(
            out=nl_o[:, :, :].rearrange("b (pp rep) c -> (b pp) rep c", pp=16),
            in_=nlx,
        )
        emit_muv_out()
        for bb in range(6, BPC):
            emit_edges_batch(bb, alt=True)

    nc.finalize()
    return nc


_NC = None


def _in_maps(inputs):
    weights = {
        k: np.ascontiguousarray(np.asarray(inputs[k], dtype=np.float32))
        for k in (
            "gnn_w1", "gnn_b1", "gnn_w2", "gnn_b2",
            "mu_w", "mu_b", "lv_w", "lv_b",
            "z_w", "z_b", "nd_w", "nd_b",
            "e1_w", "e1_b", "e2_w", "e2_b",
        )
    }
    nf = np.asarray(inputs["node_features"], dtype=np.float32)
    eps = np.asarray(inputs["eps"], dtype=np.float32)

    in_maps = []
    for i in range(NCORES):
        m = dict(weights)
        m["node_features"] = np.ascontiguousarray(
            nf[i * BPC : (i + 1) * BPC].reshape(BPC * N, H).T
        ).astype(ml_dtypes.bfloat16)
        m["eps"] = np.ascontiguousarray(eps[i * BPC : (i + 1) * BPC])
        in_maps.append(m)
    return in_maps


def kernel(**inputs):
    global _NC
    if _NC is None:
        _NC = _build()
    nc = _NC

    res = run_bass_kernel_spmd(nc, _in_maps(inputs), core_ids=list(range(NCORES)))
    outs = res.results
    node_logits = np.concatenate([o["node_logits"] for o in outs], axis=0)
    edge_probs = np.concatenate([o["edge_probs"] for o in outs], axis=0)
    mu = np.concatenate([o["mu"] for o in outs], axis=0)
    logvar = np.concatenate([o["logvar"] for o in outs], axis=0)
    return node_logits, edge_probs, mu, logvar


# revision 38
# speedup vs baseline: 1.1301x; 1.1301x over previous
"""Trainium2 Bass kernel for the DVAE problem.

Full-input contract: kernel(**inputs) takes the complete (unsharded) numpy
inputs and returns (node_logits, edge_probs, mu, logvar) as full numpy arrays.
Internally shards batch 64 -> 8 cores (data parallel), weights replicated.

Math (see reference):
  h      = relu(x @ W1 + b1) @ W2 + b2          [B,N,H]
  hp     = mean(h, axis=1)                      [B,H]
  mu/lv  = hp @ {mu_w,lv_w} + {mu_b,lv_b}       [B,L]
  z      = mu + eps * exp(0.5 lv)
  hd     = relu(z @ z_w + z_b)                  [B,H]
  nlrow  = hd @ nd_w + nd_b                     [B,T]  (broadcast over N)
  e      = sigmoid(relu([hd,hd] @ e1_w + e1_b) @ e2_w + e2_b)   [B]
  edge   = e[:,None,None] * strict_lower(N)     [B,N,N]

Key restructurings:
  - adj_matrix is unused by the math: never shipped or read.
  - mean commutes with the second GNN linear: pool relu(xW1+b1) first, then
    a tiny [B,H]x[H,H] matmul. The 1/N mean is folded into the relu's scale
    (relu is positively homogeneous), with b1 pre-scaled by 1/N.
  - GEMM computed transposed ([h, rows] tiles) so bias is per-partition and
    the node-pool is a free ACT accum_out (relu written in place to PSUM
    and discarded).
  - All matmuls run in fp32r: 1 cyc/row at N=512 vs 4 for fp32 (no HI/LO
    double pass), with operands rounded on load (cast-DMA) or by the
    producing compute op, as the BIR verifier requires.
  - edge_probs is split into an upper-triangle ZEROS pre-pass (independent
    of e, written during the load ramp) and ragged lower-trapezoid writes
    after each decoder chain, shrinking the serial tail.
  - Decoder chain (per 4 batches, rhs N=4) and edge writes of quad q are
    interleaved between quad q+1's GEMM windows so the in-order PE/DVE
    streams fill each other's dependency gaps.
"""

import sys

sys.path.insert(0, "/opt/trn_rl_repo")

from contextlib import ExitStack

import ml_dtypes
import numpy as np

import concourse.bass as bass
from concourse import bacc
import concourse.mybir as mybir
import concourse.tile as tile
from concourse.bass import ds
from concourse.bass_utils import run_bass_kernel_spmd
from concourse.masks import make_identity

F32 = mybir.dt.float32
F32R = mybir.dt.float32r
BF16 = mybir.dt.bfloat16
AF = mybir.ActivationFunctionType

B, N, H, L, T = 64, 1024, 512, 256, 32
NCORES = 8
BPC = B // NCORES  # batches per core = 8
NQUADS = BPC // 4  # chain granularity: 4 batches


def _build():
    nc = bacc.Bacc(None, target_bir_lowering=False)

    x_i = nc.declare_dram_parameter("node_features", [H, BPC * N], BF16, isOutput=False)
    eps_i = nc.declare_dram_parameter("eps", [BPC, L], F32, isOutput=False)
    w1_i = nc.declare_dram_parameter("gnn_w1", [H, H], F32, isOutput=False)
    b1_i = nc.declare_dram_parameter("gnn_b1", [H], F32, isOutput=False)
    w2_i = nc.declare_dram_parameter("gnn_w2", [H, H], F32, isOutput=False)
    b2_i = nc.declare_dram_parameter("gnn_b2", [H], F32, isOutput=False)
    muw_i = nc.declare_dram_parameter("mu_w", [H, L], F32, isOutput=False)
    mub_i = nc.declare_dram_parameter("mu_b", [L], F32, isOutput=False)
    lvw_i = nc.declare_dram_parameter("lv_w", [H, L], F32, isOutput=False)
    lvb_i = nc.declare_dram_parameter("lv_b", [L], F32, isOutput=False)
    zw_i = nc.declare_dram_parameter("z_w", [L, H], F32, isOutput=False)
    zb_i = nc.declare_dram_parameter("z_b", [H], F32, isOutput=False)
    ndw_i = nc.declare_dram_parameter("nd_w", [H, T], F32, isOutput=False)
    ndb_i = nc.declare_dram_parameter("nd_b", [T], F32, isOutput=False)
    e1w_i = nc.declare_dram_parameter("e1_w", [2 * H, H], F32, isOutput=False)
    e1b_i = nc.declare_dram_parameter("e1_b", [H], F32, isOutput=False)
    e2w_i = nc.declare_dram_parameter("e2_w", [H, 1], F32, isOutput=False)
    e2b_i = nc.declare_dram_parameter("e2_b", [1], F32, isOutput=False)

    nl_o = nc.declare_dram_parameter("node_logits", [BPC, N, T], F32, isOutput=True)
    ep_o = nc.declare_dram_parameter("edge_probs", [BPC, N, N], F32, isOutput=True)
    mu_o = nc.declare_dram_parameter("mu", [BPC, L], F32, isOutput=True)
    lv_o = nc.declare_dram_parameter("logvar", [BPC, L], F32, isOutput=True)


    with tile.TileContext(nc) as tc, ExitStack() as ctx:
        singles = ctx.enter_context(tc.tile_pool(name="singles", bufs=1))
        xTw = ctx.enter_context(tc.tile_pool(name="xTw", bufs=3))
        epool = ctx.enter_context(tc.tile_pool(name="epool", bufs=3))
        chpool = ctx.enter_context(tc.tile_pool(name="chain", bufs=4))
        outp = ctx.enter_context(tc.tile_pool(name="outp", bufs=2))
        psA = ctx.enter_context(tc.tile_pool(name="psA", bufs=4, space="PSUM"))
        psC = ctx.enter_context(tc.tile_pool(name="psC", bufs=2, space="PSUM"))

        # ---- critical-path prologue: W1, b1, first x windows ----
        w1sb = singles.tile([128, 4, H], BF16)
        nc.gpsimd.dma_start(out=w1sb, in_=w1_i[:, :].rearrange("(c p) h -> p c h", p=128))
        b1sb = singles.tile([128, 4], F32)
        nc.scalar.dma_start(out=b1sb, in_=b1_i[:].rearrange("(c p) -> p c", p=128))
        nc.scalar.mul(b1sb, b1sb, 1.0 / N)  # relu scale folds the 1/N mean
        # PE warm-up: dummy matmuls over the just-loaded W1 keep the HAM
        # activity window busy while x streams in, so real GEMM work starts
        # at 2.4 GHz instead of the cold 1.2 GHz.
        pswarm = psA.tile([128, 512], F32, tag="psa")
        for i in range(16):
            nc.tensor.matmul(
                pswarm,
                w1sb[:, i % 4, ds(0, 128)],
                w1sb[:, i % 4, :],
                start=True,
                stop=True,
            )

        # masks/zeros memsets early (DVE is idle during the ramp)
        masks = singles.tile([128, 8, N], BF16)
        nc.vector.memset(masks, 1.0)
        ones128 = singles.tile([128, 128], F32)
        nc.vector.memset(ones128, 1.0)

        # persistent accumulators / staging (declared early, used throughout)
        accall = singles.tile([128, NG := 16, 4], F32)
        muall = singles.tile([128, 2, BPC], F32)
        lvall = singles.tile([128, 2, BPC], F32)
        ndall = singles.tile([128, 32], F32)
        eall = singles.tile([128, BPC], F32)

        # ---- weight folds (emitted after the bulk prologue)  ----
        # mu = (hp@W2+b2)@mu_w+mu_b = hp@(W2@mu_w) + (b2@mu_w+mu_b); same for
        # lv. he = relu([hd,hd]@e1_w+e1_b) = relu(hd@(e1_w[:H]+e1_w[H:])+e1_b).
        def emit_fold_setup():
            nonlocal w2mu, w2lv, e1f
            w2mu = singles.tile([128, 4, L], F32R)
            w2lv = singles.tile([128, 4, L], F32R)
            e1f = singles.tile([128, 4, H], F32R)
            for c in range(4):
                nc.vector.tensor_add(
                    e1f[:, c, :], e1wsb[:, c, :], e1wsb[:, c + 4, :]
                )
            # W2^T via PE transposes (W2 viewed as f32; rounding on the copy)
            w2T = singles.tile([128, 4, H], F32R)
            for kc in range(4):
                pst = psC.tile([128, 4, 128], F32, tag="chps")
                for hc in range(4):
                    nc.tensor.transpose(
                        pst[:, hc, :],
                        w2sb[:, hc, ds(128 * kc, 128)].bitcast(F32),
                        ident,
                    )
                nc.vector.tensor_copy(w2T[:, kc, :].rearrange("p (c f) -> p c f", c=4), pst)
            for tgt, wsb in ((w2mu, muwsb), (w2lv, lvwsb)):
                for hc in range(4):
                    ps = psC.tile([128, L], F32, tag="chps")
                    for k in range(4):
                        nc.tensor.matmul(
                            ps,
                            w2T[:, k, ds(128 * hc, 128)],
                            wsb[:, k, :],
                            start=(k == 0),
                            stop=(k == 3),
                        )
                    nc.vector.tensor_copy(tgt[:, hc, :], ps)

        w2mu = w2lv = e1f = None

        # ---- deferred bulk prologue (emitted between early GEMM windows) ----
        def emit_bulk_prologue():
            nonlocal w2sb, muwsb, lvwsb, zwsb, ndwsb, e1wsb, e2cols
            nonlocal b2sb, mubsb, lvbsb, zbsb, e1bsb, ndbsb, e2bsb, epsT
            nonlocal e2rep, selT, ident
            e2rep = singles.tile([128, 4, 128], F32R)
            selT = singles.tile([128, 128], F32R)
            ident = singles.tile([128, 128], F32)
            w2sb = singles.tile([128, 4, H], F32R)
            nc.gpsimd.dma_start(
                out=w2sb, in_=w2_i[:, :].rearrange("(c p) h -> p c h", p=128)
            )
            muwsb = singles.tile([128, 4, L], F32R)
            nc.gpsimd.dma_start(
                out=muwsb, in_=muw_i[:, :].rearrange("(c p) l -> p c l", p=128)
            )
            lvwsb = singles.tile([128, 4, L], F32R)
            nc.gpsimd.dma_start(
                out=lvwsb, in_=lvw_i[:, :].rearrange("(c p) l -> p c l", p=128)
            )
            zwsb = singles.tile([128, 2, H], F32R)
            nc.gpsimd.dma_start(
                out=zwsb, in_=zw_i[:, :].rearrange("(c p) h -> p c h", p=128)
            )
            ndwsb = singles.tile([128, 4, T], F32R)
            nc.gpsimd.dma_start(
                out=ndwsb, in_=ndw_i[:, :].rearrange("(c p) t -> p c t", p=128)
            )
            e1wsb = singles.tile([128, 8, H], F32R)
            nc.gpsimd.dma_start(
                out=e1wsb, in_=e1w_i[:, :].rearrange("(c p) h -> p c h", p=128)
            )
            b2sb = singles.tile([128, 4], F32R)
            nc.gpsimd.dma_start(out=b2sb, in_=b2_i[:].rearrange("(c p) -> p c", p=128))
            mubsb = singles.tile([128, 2], F32)
            nc.scalar.dma_start(out=mubsb, in_=mub_i[:].rearrange("(c p) -> p c", p=128))
            lvbsb = singles.tile([128, 2], F32)
            nc.scalar.dma_start(out=lvbsb, in_=lvb_i[:].rearrange("(c p) -> p c", p=128))
            zbsb = singles.tile([128, 4], F32)
            nc.scalar.dma_start(out=zbsb, in_=zb_i[:].rearrange("(c p) -> p c", p=128))
            e1bsb = singles.tile([128, 4], F32)
            nc.scalar.dma_start(out=e1bsb, in_=e1b_i[:].rearrange("(c p) -> p c", p=128))
            ndbsb = singles.tile([128, 1], F32)
            nc.scalar.dma_start(
                out=ndbsb[0:T, :], in_=ndb_i[:].rearrange("(t o) -> t o", o=1)
            )
            e2bsb = singles.tile([128, 1], F32)
            nc.gpsimd.dma_start(
                out=e2bsb, in_=bass.AP(tensor=e2b_i, offset=0, ap=[[0, 128], [1, 1]])
            )
            epsT = singles.tile([128, 2, BPC], F32)
            for c in range(2):
                nc.scalar.dma_start(
                    out=epsT[:, c, :],
                    in_=eps_i[:, ds(128 * c, 128)].rearrange("b l -> l b"),
                )
            nc.vector.memset(ndall, 0.0)
            for r in range(8):
                nc.gpsimd.affine_select(
                    out=masks[:, r, :],
                    in_=masks[:, r, :],
                    compare_op=mybir.AluOpType.is_gt,
                    fill=0.0,
                    base=128 * r,
                    channel_multiplier=1,
                    pattern=[[-1, N]],
                )
            make_identity(nc, ident)
            # selection matrix: selT[b, p] = 1 iff p//16 == b
            selS = singles.tile([128, 128], F32)
            nc.vector.memset(selS, 1.0)
            nc.gpsimd.affine_select(
                out=selS[0:BPC, :], in_=selS[0:BPC, :],
                compare_op=mybir.AluOpType.is_ge, fill=0.0,
                base=0, channel_multiplier=-16, pattern=[[1, 128]],
            )
            nc.gpsimd.affine_select(
                out=selS[0:BPC, :], in_=selS[0:BPC, :],
                compare_op=mybir.AluOpType.is_gt, fill=0.0,
                base=16, channel_multiplier=16, pattern=[[-1, 128]],
            )
            nc.vector.tensor_copy(selT[0:BPC, :], selS[0:BPC, :])
            # e2 weight replicated across all 128 stationary columns so the
            # final sigmoid lands on every partition (no partition broadcast)
            e2f = singles.tile([128, 4], F32)
            nc.scalar.dma_start(
                out=e2f, in_=e2w_i[:, :].rearrange("(c p) o -> p (c o)", p=128)
            )
            for k in range(4):
                nc.vector.tensor_scalar_mul(
                    e2rep[:, k, :], ones128, e2f[:, ds(k, 1)]
                )

        w2sb = muwsb = lvwsb = zwsb = ndwsb = e1wsb = e2cols = None
        b2sb = mubsb = lvbsb = zbsb = e1bsb = ndbsb = e2bsb = epsT = None
        e2rep = selT = ident = None

        # GEMM window: 1024 rows (2 row-groups); xT loaded directly from
        # the host-transposed input, then W1 matmuls with fused
        # relu(+bias, +1/N)+row-sum into accall.
        def emit_gemm_window(q, hl):
            rbase = 4096 * q + 1024 * hl
            xT = xTw.tile([128, 4, 1024], BF16)
            nc.sync.dma_start(
                out=xT,
                in_=x_i[:, ds(rbase, 1024)].rearrange("(c p) r -> p c r", p=128),
            )
            for sub in range(2):
                gidx = 8 * q + 2 * hl + sub
                for m in range(4):
                    psa = psA.tile([128, 512], F32, tag="psa")
                    for c in range(4):
                        nc.tensor.matmul(
                            psa,
                            w1sb[:, c, ds(128 * m, 128)],
                            xT[:, c, ds(512 * sub, 512)],
                            start=(c == 0),
                            stop=(c == 3),
                        )
                    nc.scalar.activation(
                        out=psa,
                        in_=psa,
                        func=AF.Relu,
                        bias=b1sb[:, ds(m, 1)],
                        scale=1.0 / N,
                        accum_out=accall[:, gidx, ds(m, 1)],
                    )

        # Decoder chain for batches 4q..4q+3, split into two emission chunks
        # so GEMM windows can be slotted between them.
        def emit_chain_a(qb, nb):
            hp2 = chpool.tile([128, 4, 4], F32R, tag="hp2")
            for m in range(4):
                for j in range(nb):
                    g0 = 2 * (qb + j)
                    nc.vector.tensor_add(
                        hp2[:, m, ds(j, 1)],
                        accall[:, g0, ds(m, 1)],
                        accall[:, g0 + 1, ds(m, 1)],
                    )
            for m in range(2):
                ps = psC.tile([128, 4], F32, tag="chps")
                for k in range(4):
                    nc.tensor.matmul(
                        ps[:, 0:nb], w2mu[:, k, ds(128 * m, 128)], hp2[:, k, 0:nb],
                        start=(k == 0), stop=(k == 3),
                    )
                nc.vector.tensor_scalar_add(
                    muall[:, m, ds(qb, nb)], ps[:, 0:nb], mubsb[:, ds(m, 1)]
                )
                ps2 = psC.tile([128, 4], F32, tag="chps")
                for k in range(4):
                    nc.tensor.matmul(
                        ps2[:, 0:nb], w2lv[:, k, ds(128 * m, 128)], hp2[:, k, 0:nb],
                        start=(k == 0), stop=(k == 3),
                    )
                nc.vector.tensor_scalar_add(
                    lvall[:, m, ds(qb, nb)], ps2[:, 0:nb], lvbsb[:, ds(m, 1)]
                )
            return hp2

        def emit_chain_b(qb, nb):
            zT = chpool.tile([128, 2, 4], F32R, tag="zT")
            for m in range(2):
                tmp = chpool.tile([128, 4], F32, tag="ztmp")
                nc.scalar.activation(
                    out=tmp[:, 0:nb], in_=lvall[:, m, ds(qb, nb)], func=AF.Exp, scale=0.5
                )
                nc.vector.tensor_mul(tmp[:, 0:nb], tmp[:, 0:nb], epsT[:, m, ds(qb, nb)])
                nc.vector.tensor_add(
                    zT[:, m, 0:nb], tmp[:, 0:nb], muall[:, m, ds(qb, nb)]
                )
            hdT = chpool.tile([128, 4, 4], F32R, tag="hdT")
            for m in range(4):
                ps = psC.tile([128, 4], F32, tag="chps")
                for k in range(2):
                    nc.tensor.matmul(
                        ps[:, 0:nb], zwsb[:, k, ds(128 * m, 128)], zT[:, k, 0:nb],
                        start=(k == 0), stop=(k == 1),
                    )
                nc.scalar.activation(
                    out=hdT[:, m, 0:nb], in_=ps[:, 0:nb], func=AF.Relu,
                    bias=zbsb[:, ds(m, 1)],
                )
            return hdT

        def emit_chain_c(qb, nb, hdT):
            psn = psC.tile([128, 4], F32, tag="chps")
            for k in range(4):
                nc.tensor.matmul(
                    psn[0:T, 0:nb], ndwsb[:, k, :], hdT[:, k, 0:nb],
                    start=(k == 0), stop=(k == 3),
                )
            nc.vector.tensor_scalar_add(
                ndall[0:T, ds(qb, nb)], psn[0:T, 0:nb], ndbsb[0:T, :]
            )
            heT = chpool.tile([128, 4, 4], F32R, tag="heT")
            for m in range(4):
                ps = psC.tile([128, 4], F32, tag="chps")
                for k in range(4):
                    nc.tensor.matmul(
                        ps[:, 0:nb], e1f[:, k, ds(128 * m, 128)], hdT[:, k, 0:nb],
                        start=(k == 0), stop=(k == 3),
                    )
                nc.scalar.activation(
                    out=heT[:, m, 0:nb], in_=ps[:, 0:nb], func=AF.Relu,
                    bias=e1bsb[:, ds(m, 1)],
                )
            pse = psC.tile([128, 4], F32, tag="chps")
            for k in range(4):
                nc.tensor.matmul(
                    pse[:, 0:nb], e2rep[:, k, :], heT[:, k, 0:nb],
                    start=(k == 0), stop=(k == 3),
                )
            nc.scalar.activation(
                out=eall[:, ds(qb, nb)], in_=pse[:, 0:nb], func=AF.Sigmoid, bias=e2bsb
            )

        # Lower-trapezoid edge writes for one batch: 256-row chunk k covers
        # columns [0, 256(k+1)) with mask-scaled values.
        def emit_edges_batch(bb, alt=False):
            for k in range(2):
                w = 512 * (k + 1)
                et = epool.tile([128, 4, w], F32, tag=f"et{k}")
                for h4 in range(4):
                    r = 4 * k + h4
                    nc.vector.tensor_scalar_mul(
                        et[:, h4, :],
                        masks[:, r, ds(0, w)],
                        eall[:, ds(bb, 1)],
                    )
                dma_eng = nc.gpsimd if (alt and k == 0) else nc.sync
                dma_eng.dma_start(
                    out=ep_o[bb, ds(512 * k, 512), ds(0, w)].rearrange(
                        "(t p) c -> p t c", p=128
                    ),
                    in_=et,
                )

        # ---------------- emission schedule ----------------
        # Chain granularity [4, 2, 2]: batches 4-5's chain only needs row
        # groups 8..11, so their edge writes overlap the last GEMM windows,
        # leaving just a 2-batch edge tail.
        emit_gemm_window(0, 0)
        emit_gemm_window(0, 1)
        emit_bulk_prologue()
        emit_gemm_window(0, 2)
        emit_fold_setup()
        emit_gemm_window(0, 3)
        emit_gemm_window(1, 0)
        emit_chain_a(0, 4)
        emit_gemm_window(1, 1)
        hdT0 = emit_chain_b(0, 4)
        emit_chain_c(0, 4, hdT0)
        emit_gemm_window(1, 2)
        emit_edges_batch(0)
        emit_chain_a(4, 2)
        hdT1 = emit_chain_b(4, 2)
        emit_chain_c(4, 2, hdT1)
        emit_gemm_window(1, 3)
        emit_edges_batch(1)
        emit_edges_batch(2)
        emit_edges_batch(3)
        emit_edges_batch(4)
        emit_chain_a(6, 2)
        hdT2 = emit_chain_b(6, 2)
        emit_chain_c(6, 2, hdT2)
        emit_edges_batch(5)

        def emit_muv_out():
            # transpose [128 l, 2, 8 b] -> [8 b, 256 l] on the PE, then one
            # clean contiguous DMA per output (the direct scatter AP would
            # cost ~1024 4-byte descriptors and stall the SDMA engines).
            for src_t, dst in ((muall, mu_o), (lvall, lv_o)):
                pso = psC.tile([128, 2, 128], F32, tag="chps")
                for c in range(2):
                    nc.tensor.transpose(pso[0:BPC, c, :], src_t[:, c, :], ident)
                row = outp.tile([128, 2, 128], F32, tag="orow")
                nc.vector.tensor_copy(row[0:BPC, :, :], pso[0:BPC, :, :])
                nc.sync.dma_start(
                    out=dst[:, :], in_=row[0:BPC, :, :].rearrange("b c l -> b (c l)")
                )

        # ---------------- tail: node_logits + mu/logvar before last edges ----
        # nd rows land on all 128 partitions (p -> batch p//16) via a 0/1
        # selection matmul; no DRAM bounce needed.
        ndR = chpool.tile([128, 32], F32)
        nc.vector.transpose(ndR[0:32, :], ndall[0:32, :])  # [b, t] rows 0..7
        ndRr = chpool.tile([128, 32], F32R)
        nc.vector.tensor_copy(ndRr[0:BPC, :], ndR[0:BPC, :])
        psl = psA.tile([128, 512], F32, tag="psa")
        nc.tensor.matmul(
            psl[:, 0:T], selT[0:BPC, :], ndRr[0:BPC, :], start=True, stop=True
        )
        nlx = singles.tile([128, 64, T], F32)
        nc.vector.tensor_copy(
            out=nlx,
            in_=bass.AP(
                tensor=psl.tensor,
                offset=psl.offset,
                ap=[psl.ap[0], [0, 64], [1, T]],
            ),
        )
        nc.sync.dma_start(
            out=nl_o[:, :, :].rearrange("b (pp rep) c -> (b pp) rep c", pp=16),
            in_=nlx,
        )
        emit_muv_out()
        for bb in range(6, BPC):
            emit_edges_batch(bb, alt=True)

    nc.finalize()
    return nc


_NC = None


def _in_maps(inputs):
    weights = {
        k: np.ascontiguousarray(np.asarray(inputs[k], dtype=np.float32))
        for k in (
            "gnn_w1", "gnn_b1", "gnn_w2", "gnn_b2",
            "mu_w", "mu_b", "lv_w", "lv_b",
            "z_w", "z_b", "nd_w", "nd_b",
            "e1_w", "e1_b", "e2_w", "e2_b",
        )
    }
    b2 = np.asarray(inputs["gnn_b2"], np.float64)
    weights["mu_b"] = np.ascontiguousarray(
        (b2 @ np.asarray(inputs["mu_w"], np.float64)
         + np.asarray(inputs["mu_b"], np.float64)).astype(np.float32))
    weights["lv_b"] = np.ascontiguousarray(
        (b2 @ np.asarray(inputs["lv_w"], np.float64)
         + np.asarray(inputs["lv_b"], np.float64)).astype(np.float32))
    nf = np.asarray(inputs["node_features"], dtype=np.float32)
    eps = np.asarray(inputs["eps"], dtype=np.float32)

    in_maps = []
    for i in range(NCORES):
        m = dict(weights)
        m["node_features"] = np.ascontiguousarray(
            nf[i * BPC : (i + 1) * BPC].reshape(BPC * N, H).T
        ).astype(ml_dtypes.bfloat16)
        m["eps"] = np.ascontiguousarray(eps[i * BPC : (i + 1) * BPC])
        in_maps.append(m)
    return in_maps


def kernel(**inputs):
    global _NC
    if _NC is None:
        _NC = _build()
    nc = _NC

    res = run_bass_kernel_spmd(nc, _in_maps(inputs), core_ids=list(range(NCORES)))
    outs = res.results
    node_logits = np.concatenate([o["node_logits"] for o in outs], axis=0)
    edge_probs = np.concatenate([o["edge_probs"] for o in outs], axis=0)
    mu = np.concatenate([o["mu"] for o in outs], axis=0)
    logvar = np.concatenate([o["logvar"] for o in outs], axis=0)
    return node_logits, edge_probs, mu, logvar
